# revision 12
# baseline (speedup 1.0000x reference)
"""DiffeomorphismNet fused kernel for 8x TRN2 NeuronCores (data parallel).

Math (per sample row x = [xt | xtdot | xz], each 64 wide):
  branch(v):  h0 = W_in v + b_in;  h_{i+1} = relu(W_h[i] h_i + b_h[i]), i=0..2
              D_{i+1} = (W_h[i] h_{i+1} + b_h[i] > 0)        # the module's quirk
  out cols  0:64   h_out  = W_out h3_t + b_out                       (t branch)
  out cols 64:128  h_dot  = W_out D3t W2 D2t W1 D1t (W_h0 W_in) xtdot
  out cols 128:192 zng    = row_norms(W_out D3z W2 D2z W1 D1z G0),  G0 = W_h0 W_in

Precision strategy (validated in a numpy bit-level sim, rel_l2 ~ 7e-3 vs fp64):
  - Forward h-value matmuls: fp32r hi pass + ONE fp8e4 DoubleRow cross pass.
    The cross PSUM holds 2^12 * (Wlo@hhi + Whi@hlo): stationary pairs
    (e4(Wlo*2^12), e4(Whi)) ride DoubleRow against moving pairs (hhi8, hlo8*2^12);
    combine as z = z_hi + 2^-12 * z_cross.  z error ~1e-5 relative.
  - Forward MASK matmuls (sign-critical): 3-pass fp32r hi/lo (err ~2e-7).
  - h0: 3-pass fp32r.  h_out: exact fp32.  h_dot chain: single-pass fp32r.
  - zng Jacobian chain: fp8e4 DoubleRow (K=256/instr = 2x fp32r MAC rate).
    Power-of-2 scales keep fp8 operands in e4m3 normal range:
    G0*8 -> J0; W1*16 -> J1 (sigma~11); W2*2 -> J2 (sigma~16); Wout*16 -> A.
    zng = sqrt(sum A^2 * 2^-24).

Engine balance in the zng phase (PE ~3.8us per group of 8 samples):
  gpsimd: J0 = e4(G0 x D1) build (scalar_tensor_tensor, SBUF-only) + d-reduce;
  DVE: 3 of 4 masked PSUM->fp8 casts per layer;
  ACT: remaining cast as 8 per-sample Copy(scale=mask[p,1]) ops + square + sqrt.

Sharding: batch 4096 -> 8 cores x 512. Weights replicated.
"""

import os
import sys

sys.path.insert(0, "/opt/trn_rl_repo")

import numpy as np
import ml_dtypes
import concourse.bass as bass
import concourse.tile as tile
from concourse import bacc
from concourse import mybir
from concourse.bass_utils import run_bass_kernel_spmd

N_CORES = 8
B = 4096
BC = B // N_CORES          # 512 samples per core
D = 64                     # n
H = 512                    # hidden
NL = 3                     # hidden layers
NMC = H // 128             # partition chunks of the hidden dim
NG = BC // 8               # jacobian groups of 8 samples
JN = 8 * D                 # 512 columns per jacobian group

F32 = mybir.dt.float32
F32R = mybir.dt.float32r
F8 = mybir.dt.float8e4
E4NP = ml_dtypes.float8_e4m3
DR = mybir.MatmulPerfMode.DoubleRow

ADD = mybir.AluOpType.add
MAX = mybir.AluOpType.max
MULT = mybir.AluOpType.mult
SUB = mybir.AluOpType.subtract
ISGT = mybir.AluOpType.is_gt
AF = mybir.ActivationFunctionType

CS = float(2.0 ** 12)      # cross-pass pre-scale
ICS = float(2.0 ** -12)
S0, S1, S2, SO = 8.0, 16.0, 2.0, 16.0     # zng chain scales (product 2^12)
ZS = float(2.0 ** -24)     # sqrt scale = 1/(S0*S1*S2*SO)^2

# cb (fp32) column offsets
_O_WOUTR = 0                 # W_out^T as [128, 4, 64]
_O_BIN = 256                 # [128, 4]
_O_BH = 260                  # [128, 12]
_O_BOUTR = 272               # [128, 64]
_O_ID = 336                  # [128, 128] identity
CB_COLS = 464
# cbr1a (fp32r): layer-0 hi weights + W_in (for G0) + W_in^T hi/lo
_O_RWT0 = 0                  # hi(W_h[0]^T)  [128, 4, 512]
_O_WINR = 2048               # hi(W_in)*S0 as [128, 4, 64]
_O_WINTH = 2304              # hi(W_in^T) rows 0:64, [64, 512]
_O_WINTL = 2816              # lo(W_in^T) rows 0:64, [64, 512]
CBR1A_COLS = 3328
# cbr1b (fp32r): layer-0 lo weights
_O_WLO0 = 0
CBR1B_COLS = 2048
# cbr2a (fp32r): layer 1
_O_RWT1 = 0
_O_WLO1 = 2048
CBR2A_COLS = 4096
# cbr2b (fp32r): layer 2 + W_out^T
_O_RWT2 = 0
_O_WLO2 = 2048
_O_RWOUT = 4096              # hi(W_out^T) as [128, 4, 64]
CBR2B_COLS = 4352
# cb8 (fp8e4): cross-pass stationary pairs + zng chain weights
_O8_WC = [0, 4096, 8192]     # per layer [128, 4kc, 2, 512]: (lo*2^12, hi)
_O8_W81 = 12288              # e4(W_h1^T * S1) [128, 4, 512]
_O8_W82 = 14336              # e4(W_h2^T * S2)
_O8_WOUT = 16384             # e4(W_out^T * SO) [128, 4, 64]
CB8_COLS = 16640


def _round_fp32r(x: np.ndarray) -> np.ndarray:
    """Round-to-nearest-even to 11 explicit mantissa bits (fp32r grid)."""
    u = x.astype(np.float32).view(np.uint32).astype(np.uint64)
    keep = np.uint64(0xFFFFF000)
    half = np.uint64(0x800)
    lsb = (u >> np.uint64(12)) & np.uint64(1)
    r = (u + half - np.uint64(1) + lsb) & keep
    return r.astype(np.uint32).view(np.float32)


def _e4(x: np.ndarray) -> np.ndarray:
    return np.ascontiguousarray(np.asarray(x, np.float32)).astype(E4NP)


def _build():
    nc = bacc.Bacc("TRN2", target_bir_lowering=False, debug=False,
                   num_devices=N_CORES)

    cb_d = nc.dram_tensor("cb", [128, CB_COLS], F32, kind="ExternalInput")
    cbr1a_d = nc.dram_tensor("cbr1a", [128, CBR1A_COLS], F32R, kind="ExternalInput")
    cbr1b_d = nc.dram_tensor("cbr1b", [128, CBR1B_COLS], F32R, kind="ExternalInput")
    cbr2a_d = nc.dram_tensor("cbr2a", [128, CBR2A_COLS], F32R, kind="ExternalInput")
    cbr2b_d = nc.dram_tensor("cbr2b", [128, CBR2B_COLS], F32R, kind="ExternalInput")
    cb8_d = nc.dram_tensor("cb8", [128, CB8_COLS], F8, kind="ExternalInput")
    xbh_d = nc.dram_tensor("xbh", [64, 3 * BC], F32R, kind="ExternalInput")
    xbl_d = nc.dram_tensor("xbl", [64, 3 * BC], F32R, kind="ExternalInput")
    out_d = nc.dram_tensor("out", [BC, 3 * D], F32, kind="ExternalOutput")

    with tile.TileContext(nc) as tc:
        with (
            tc.tile_pool(name="const", bufs=1) as pc,
            tc.tile_pool(name="act", bufs=2) as pa,
            tc.tile_pool(name="mask", bufs=1) as pm,
            tc.tile_pool(name="jac", bufs=3) as pj,
            tc.tile_pool(name="ps", bufs=2, space="PSUM") as ps,
            tc.tile_pool(name="psc", bufs=1, space="PSUM") as psc,
            tc.tile_pool(name="psa", bufs=1, space="PSUM") as psa,
            tc.tile_pool(name="pso", bufs=1, space="PSUM") as pso,
            tc.tile_pool(name="psd", bufs=1, space="PSUM") as psd,
        ):
            cb = pc.tile([128, CB_COLS], F32)
            cbr1a = pc.tile([128, CBR1A_COLS], F32R)
            cbr1b = pc.tile([128, CBR1B_COLS], F32R)
            cbr2a = pc.tile([128, CBR2A_COLS], F32R)
            cbr2b = pc.tile([128, CBR2B_COLS], F32R)
            cb8 = pc.tile([128, CB8_COLS], F8)
            xbh = pc.tile([64, 3 * BC], F32R)
            xbl = pc.tile([64, 3 * BC], F32R)
            nc.sync.dma_start(cbr1a[:], cbr1a_d.ap())
            nc.sync.dma_start(cb[:], cb_d.ap())
            nc.sync.dma_start(xbh[:], xbh_d.ap())
            nc.sync.dma_start(xbl[:], xbl_d.ap())
            nc.sync.dma_start(cbr1b[:], cbr1b_d.ap())
            nc.sync.dma_start(cbr2a[:], cbr2a_d.ap())
            nc.sync.dma_start(cbr2b[:], cbr2b_d.ap())
            nc.sync.dma_start(cb8[:], cb8_d.ap())

            # DVE warm-up: observe each input DMA semaphore once.
            warm = pc.tile([128, 4], F32)
            warm8 = pc.tile([128, 4], F8)
            nc.vector.tensor_copy(warm[0:1, 0:1], cb[0:1, 0:1])
            nc.vector.tensor_copy(warm[0:1, 1:2].bitcast(F32R), cbr1a[0:1, 0:1])
            nc.vector.tensor_copy(warm[0:1, 1:2].bitcast(F32R), cbr1b[0:1, 0:1])
            nc.vector.tensor_copy(warm[0:1, 2:3].bitcast(F32R), cbr2a[0:1, 0:1])
            nc.vector.tensor_copy(warm[0:1, 3:4].bitcast(F32R), cbr2b[0:1, 0:1])
            nc.vector.tensor_copy(warm[0:1, 0:1].bitcast(F32R), xbh[0:1, 0:1])
            nc.vector.tensor_copy(warm[0:1, 1:2].bitcast(F32R), xbl[0:1, 0:1])
            nc.vector.tensor_copy(warm8[0:1, 0:1], cb8[0:1, 0:1])

            WoutR = cb[:, _O_WOUTR:_O_WOUTR + 256].rearrange(
                "p (kc n) -> p kc n", kc=4)
            bin_ = cb[:, _O_BIN:_O_BIN + 4]
            bh = cb[:, _O_BH:_O_BH + 12]
            boutR = cb[:, _O_BOUTR:_O_BOUTR + 64]
            ident = cb[:, _O_ID:_O_ID + 128]

            def r3(ap, off):
                return ap[:, off:off + 2048].rearrange("p (kc m) -> p kc m", kc=4)

            WHI = [r3(cbr1a, _O_RWT0), r3(cbr2a, _O_RWT1), r3(cbr2b, _O_RWT2)]
            WLO = [r3(cbr1b, _O_WLO0), r3(cbr2a, _O_WLO1), r3(cbr2b, _O_WLO2)]
            WinR = cbr1a[:, _O_WINR:_O_WINR + 256].rearrange(
                "p (kc n) -> p kc n", kc=4)
            RWout = cbr2b[:, _O_RWOUT:_O_RWOUT + 256].rearrange(
                "p (kc n) -> p kc n", kc=4)
            WinTH = cbr1a[0:64, _O_WINTH:_O_WINTH + 512]
            WinTL = cbr1a[0:64, _O_WINTL:_O_WINTL + 512]
            WC = [cb8[:, o:o + 4096].rearrange("p (kc t m) -> p kc t m",
                                               kc=4, t=2) for o in _O8_WC]
            W81 = cb8[:, _O8_W81:_O8_W81 + 2048].rearrange(
                "p (kc m) -> p kc m", kc=4)
            W82 = cb8[:, _O8_W82:_O8_W82 + 2048].rearrange(
                "p (kc m) -> p kc m", kc=4)
            Wout8 = cb8[:, _O8_WOUT:_O8_WOUT + 256].rearrange(
                "p (kc n) -> p kc n", kc=4)

            xtTh, xtTl = xbh[:, 0:BC], xbl[:, 0:BC]
            xdTh = xbh[:, BC:2 * BC]
            xzTh, xzTl = xbh[:, 2 * BC:3 * BC], xbl[:, 2 * BC:3 * BC]

            # PE warm-up: observe the cbr1a DMA semaphore from a fresh PSUM slot
            pwarm = ps.tile([128, D], F32, tag="psj1")
            nc.tensor.matmul(pwarm[:], WHI[0][:, 0, 0:128], WHI[0][:, 0, 0:D],
                             start=True, stop=True)

            # ---- G0 = (W_h[0] @ W_in) * S0  [512, 64] as [128, 4(mc), 64] ----
            G0 = pc.tile([128, 4, D], F32)
            for mc in range(NMC):
                pg = pso.tile([128, D], F32, tag="po")
                for kc in range(NMC):
                    nc.tensor.matmul(pg[:], WHI[0][:, kc, mc * 128:(mc + 1) * 128],
                                     WinR[:, kc, :], start=(kc == 0),
                                     stop=(kc == NMC - 1))
                nc.vector.tensor_copy(G0[:, mc, :], pg[:])

            def mm3(psum, i, mc, hhi, hlo):
                """z[mc] += W_h[i] @ h via 3 fp32r passes (12 matmuls)."""
                sl = slice(mc * 128, (mc + 1) * 128)
                n = 0
                for wop, rop in ((WHI[i], hhi), (WLO[i], hhi), (WHI[i], hlo)):
                    for kc in range(NMC):
                        nc.tensor.matmul(psum[:], wop[:, kc, sl], rop[:, kc, :],
                                         start=(n == 0), stop=(n == 11))
                        n += 1

            def forward(xTh, xTl, tagpfx, keep_h3=False):
                """Forward pass; returns (h3_f32|None, masks [D1,D2,D3])."""
                hhi = pa.tile([128, 4, BC], F32R, tag="hhi", name=f"h0h{tagpfx}")
                hlo = pa.tile([128, 4, BC], F32R, tag="hlo", name=f"h0l{tagpfx}")
                h8 = pa.tile([128, 4, 2, BC], F8, tag="h8", name=f"h08{tagpfx}")
                # ---- h0 (3-pass fp32r, no relu) ----
                for mc in range(NMC):
                    p0 = ps.tile([128, BC], F32, tag="psj1", name=f"p0{tagpfx}")
                    sl = slice(mc * 128, (mc + 1) * 128)
                    for n, (w, r) in enumerate(((WinTH, xTh), (WinTL, xTh),
                                                (WinTH, xTl))):
                        nc.tensor.matmul(p0[:], w[:, sl], r,
                                         start=(n == 0), stop=(n == 2))
                    hf = pa.tile([128, BC], F32, tag="hf", name=f"hf0{tagpfx}")
                    nc.vector.tensor_scalar_add(hf[:], p0[:], bin_[:, mc:mc + 1])
                    nc.vector.tensor_copy(hhi[:, mc, :], hf[:])
                    nc.vector.tensor_tensor(hlo[:, mc, :], hf[:],
                                            hhi[:, mc, :].bitcast(F32), SUB)
                    nc.scalar.copy(h8[:, mc, 0, :], hf[:])
                    nc.scalar.activation(h8[:, mc, 1, :],
                                         hlo[:, mc, :].bitcast(F32),
                                         AF.Copy, scale=CS)
                h3f = None
                masks = []
                for i in range(NL):
                    last = (i == NL - 1)
                    hhin = pa.tile([128, 4, BC], F32R, tag="hhi",
                                   name=f"h{i+1}h{tagpfx}")
                    hlon = pa.tile([128, 4, BC], F32R, tag="hlo",
                                   name=f"h{i+1}l{tagpfx}")
                    h8n = None if last else pa.tile(
                        [128, 4, 2, BC], F8, tag="h8", name=f"h{i+1}8{tagpfx}")
                    if last and keep_h3:
                        h3f = pa.tile([128, 4, BC], F32, tag="h3f", name="h3f")
                    for mc in range(NMC):
                        sl = slice(mc * 128, (mc + 1) * 128)
                        # fp32r hi pass
                        pz = ps.tile([128, BC], F32, tag="psj1", name=f"pz{tagpfx}")
                        for kc in range(NMC):
                            nc.tensor.matmul(pz[:], WHI[i][:, kc, sl],
                                             hhi[:, kc, :], start=(kc == 0),
                                             stop=(kc == NMC - 1))
                        # fp8 DoubleRow cross pass (2^12-scaled)
                        pcx = psc.tile([128, BC], F32, tag="pc",
                                       name=f"pc{tagpfx}")
                        for kc in range(NMC):
                            nc.tensor.matmul(pcx[:], WC[i][:, kc, :, sl],
                                             h8[:, kc, :, :], start=(kc == 0),
                                             stop=(kc == NMC - 1), perf_mode=DR)
                        c1 = pa.tile([128, BC], F32, tag="c1", name=f"c1{tagpfx}")
                        nc.scalar.activation(c1[:], pcx[:], AF.Copy, scale=ICS)
                        # t = (z_hi + b) + 2^-12 z_cross   (pre-relu)
                        hf = pa.tile([128, BC], F32, tag="hf", name=f"hf{tagpfx}")
                        nc.vector.scalar_tensor_tensor(
                            hf[:], pz[:], bh[:, 4 * i + mc:4 * i + mc + 1],
                            c1[:], ADD, ADD)
                        # splits of h_{i+1} = relu(t)
                        nc.vector.tensor_scalar_max(hhin[:, mc, :], hf[:], 0.0)
                        nc.vector.scalar_tensor_tensor(
                            hlon[:, mc, :], hf[:], 0.0,
                            hhin[:, mc, :].bitcast(F32), MAX, SUB)
                        if h8n is not None:
                            nc.scalar.activation(h8n[:, mc, 0, :], hf[:], AF.Relu)
                            nc.scalar.activation(h8n[:, mc, 1, :],
                                                 hlon[:, mc, :].bitcast(F32),
                                                 AF.Copy, scale=CS)
                        if h3f is not None:
                            nc.scalar.activation(h3f[:, mc, :], hf[:], AF.Relu)
                    hhi, hlo, h8 = hhin, hlon, h8n
                    # mask: 3-pass fp32r on h_{i+1}
                    Dm = pm.tile([128, 4, BC], F32, tag=f"D{i}",
                                 name=f"D{i}{tagpfx}")
                    for mc in range(NMC):
                        pd = ps.tile([128, BC], F32, tag="psj1", name=f"pd{tagpfx}")
                        mm3(pd, i, mc, hhi, hlo)
                        nc.vector.tensor_scalar(
                            Dm[:, mc, :], pd[:],
                            bh[:, 4 * i + mc:4 * i + mc + 1], 0.0, ADD, ISGT)
                    masks.append(Dm)
                return h3f, masks

            # ---- output staging tiles, one per 128-sample block ----
            O = [pc.tile([128, 3 * D], F32, tag=f"O{g}", name=f"O{g}")
                 for g in range(4)]

            # ================= t branch =================
            h3t, Dt = forward(xtTh, xtTl, "t", keep_h3=True)

            # h_out (sample-major, exact fp32): out[s,n] = h3t^T W_out^T + b_out
            for mg in range(4):
                po = pso.tile([128, D], F32, tag="po")
                for kc in range(NMC):
                    nc.tensor.matmul(po[:],
                                     h3t[:, kc, mg * 128:(mg + 1) * 128],
                                     WoutR[:, kc, :], start=(kc == 0),
                                     stop=(kc == NMC - 1))
                nc.vector.tensor_add(O[mg][:, 0:D], po[:], boutR)

            # h_dot chain, single-pass fp32r: v = W_h0 (W_in xdT); v = Di*(W v)
            w0r = pa.tile([128, 4, BC], F32R, tag="hhi", name="w0r")
            for mc in range(NMC):
                pw = ps.tile([128, BC], F32, tag="psj1", name="pw")
                nc.tensor.matmul(pw[:], WinTH[:, mc * 128:(mc + 1) * 128], xdTh,
                                 start=True, stop=True)
                nc.vector.tensor_copy(w0r[:, mc, :], pw[:])
            v = w0r
            for i in range(NL):
                vn = pa.tile([128, 4, BC], F32R, tag="hlo", name=f"v{i+1}")
                for mc in range(NMC):
                    pv = ps.tile([128, BC], F32, tag="psj1", name="pv")
                    for kc in range(NMC):
                        nc.tensor.matmul(pv[:],
                                         WHI[i][:, kc, mc * 128:(mc + 1) * 128],
                                         v[:, kc, :], start=(kc == 0),
                                         stop=(kc == NMC - 1))
                    nc.vector.tensor_mul(vn[:, mc, :], pv[:], Dt[i][:, mc, :])
                v = vn
            for mg in range(4):
                po = pso.tile([128, D], F32, tag="po")
                for kc in range(NMC):
                    nc.tensor.matmul(po[:], v[:, kc, mg * 128:(mg + 1) * 128],
                                     RWout[:, kc, :], start=(kc == 0),
                                     stop=(kc == NMC - 1))
                nc.vector.tensor_copy(O[mg][:, D:2 * D], po[:])

            # ================= z branch =================
            _, Dz = forward(xzTh, xzTl, "z")

            zng2 = pc.tile([64, BC], F32)
            zngT = pc.tile([64, BC], F32)

            def zng_flush(mg):
                c0 = mg * 128
                nc.scalar.activation(zngT[:, c0:c0 + 128], zng2[:, c0:c0 + 128],
                                     AF.Sqrt, scale=ZS)
                pt = pso.tile([128, 64], F32, tag="po")
                nc.tensor.transpose(pt[:], zngT[:, c0:c0 + 128],
                                    ident[0:64, 0:64])
                nc.vector.tensor_copy(O[mg][:, 2 * D:3 * D], pt[:])

            # ============ zng: fp8 DoubleRow Jacobian chain ============
            # PE keep-warm: dependency-free matmuls that execute inside
            # cross-engine wait windows, preventing p-state down-clocking.
            pdum = psd.tile([128, BC], F32, tag="pd")

            def warmmm(n=1):
                for _ in range(n):
                    nc.tensor.matmul(pdum[:], WHI[0][:, 0, 0:128],
                                     WHI[0][:, 1, :], start=True, stop=True,
                                     skip_group_check=True)

            # Software-pipelined across groups: stage s processes group g-s,
            # giving each cross-engine dependency a full iteration of slack
            # (the in-order PE queue otherwise stalls on trailing casts).
            J0r, J1r, J2r, pAr, sqr = {}, {}, {}, {}, {}

            def st_j0(g):
                s0 = g * 8
                J0 = pj.tile([128, 4, 8, D], F8, tag="J0", name="J0")
                nc.gpsimd.tensor_tensor(
                    J0[:],
                    G0[:, :, None, :].broadcast_to([128, 4, 8, D]),
                    Dz[0][:, :, s0:s0 + 8][:, :, :, None]
                    .broadcast_to([128, 4, 8, D]), MULT)
                J0r[g] = J0
                warmmm(1)

            def st_layer(g, i, W8, Jsrc, Jdst):
                s0 = g * 8
                J = Jsrc.pop(g)
                Jn = pj.tile([128, 4, 8, D], F8, tag=f"J{i}", name=f"J{i}")
                for mc in range(NMC):
                    sl = slice(mc * 128, (mc + 1) * 128)
                    pjm = ps.tile([128, JN], F32, tag=f"psj{i}", name="pjm")
                    for kp in range(2):
                        nc.tensor.matmul(
                            pjm[:], W8[:, 2 * kp:2 * kp + 2, sl],
                            J[:, 2 * kp:2 * kp + 2, :, :],
                            start=(kp == 0), stop=(kp == 1), perf_mode=DR)
                    if mc == 2 and i == 2:
                        for b in range(8):
                            nc.scalar.activation(
                                Jn[:, mc, b, :],
                                pjm[:, b * D:(b + 1) * D], AF.Copy,
                                scale=Dz[i][:, mc, s0 + b:s0 + b + 1])
                    else:
                        nc.vector.tensor_tensor(
                            Jn[:, mc, :, :],
                            pjm[:].rearrange("p (b d) -> p b d", b=8),
                            Dz[i][:, mc, s0:s0 + 8][:, :, None]
                            .broadcast_to([128, 8, D]), MULT)
                Jdst[g] = Jn
                warmmm(2)

            def st_tail(g):
                s0 = g * 8
                J = J2r.pop(g)
                pA = psa.tile([64, JN], F32, tag="pA")
                for kp in range(2):
                    nc.tensor.matmul(pA[:], Wout8[:, 2 * kp:2 * kp + 2, :],
                                     J[:, 2 * kp:2 * kp + 2, :, :],
                                     start=(kp == 0), stop=(kp == 1),
                                     perf_mode=DR)
                sq = pa.tile([64, JN], F32, tag="sq", name="sq")
                nc.scalar.square(sq[:], pA[:])
                nc.vector.tensor_reduce(
                    zng2[:, s0:s0 + 8],
                    sq[:].rearrange("p (b d) -> p b d", b=8),
                    mybir.AxisListType.X, mybir.AluOpType.add)
                warmmm(2)
                if g % 16 == 15:
                    zng_flush(g // 16)

            for g in range(NG + 3):
                if g < NG:
                    st_j0(g)
                if 1 <= g < NG + 1:
                    st_layer(g - 1, 1, W81, J0r, J1r)
                if 2 <= g < NG + 2:
                    st_layer(g - 2, 2, W82, J1r, J2r)
                if 3 <= g:
                    st_tail(g - 3)

            for mg in range(4):
                nc.sync.dma_start(out_d.ap()[mg * 128:(mg + 1) * 128, :], O[mg][:])

    nc.compile()
    return nc


def _pack_consts(W_in, b_in, W_h, b_h, W_out, b_out):
    cb = np.zeros((128, CB_COLS), dtype=np.float32)
    cb[:, _O_WOUTR:_O_WOUTR + 256] = (
        W_out.T.reshape(4, 128, D).transpose(1, 0, 2).reshape(128, 256))
    cb[:, _O_BIN:_O_BIN + 4] = b_in.reshape(4, 128).T
    cb[:, _O_BH:_O_BH + 12] = b_h.reshape(3, 4, 128).transpose(2, 0, 1).reshape(128, 12)
    cb[:, _O_BOUTR:_O_BOUTR + 64] = np.tile(b_out, (128, 1))
    cb[:, _O_ID:_O_ID + 128] = np.eye(128, dtype=np.float32)

    # W_h[i]^T rearranged to [128, 4, 512] then hi/lo split on the fp32r grid
    WT = np.transpose(W_h, (0, 2, 1)).reshape(3, 4, 128, H).transpose(0, 2, 1, 3)
    WT = WT.reshape(3, 128, 2048)
    WT_hi = _round_fp32r(WT)
    WT_lo = _round_fp32r(WT - WT_hi)

    winr = W_in.reshape(4, 128, D).transpose(1, 0, 2).reshape(128, 256)
    winr = _round_fp32r(winr * S0)
    woutr = W_out.T.reshape(4, 128, D).transpose(1, 0, 2).reshape(128, 256)
    woutr = _round_fp32r(woutr)

    wint = W_in.T
    wint_hi = _round_fp32r(wint)
    wint_lo = _round_fp32r(wint - wint_hi)

    cbr1a = np.zeros((128, CBR1A_COLS), dtype=np.float32)
    cbr1a[:, _O_RWT0:_O_RWT0 + 2048] = WT_hi[0]
    cbr1a[:, _O_WINR:_O_WINR + 256] = winr
    cbr1a[0:64, _O_WINTH:_O_WINTH + 512] = wint_hi
    cbr1a[0:64, _O_WINTL:_O_WINTL + 512] = wint_lo
    cbr1b = np.ascontiguousarray(WT_lo[0])

    cbr2a = np.zeros((128, CBR2A_COLS), dtype=np.float32)
    cbr2a[:, _O_RWT1:_O_RWT1 + 2048] = WT_hi[1]
    cbr2a[:, _O_WLO1:_O_WLO1 + 2048] = WT_lo[1]
    cbr2b = np.zeros((128, CBR2B_COLS), dtype=np.float32)
    cbr2b[:, _O_RWT2:_O_RWT2 + 2048] = WT_hi[2]
    cbr2b[:, _O_WLO2:_O_WLO2 + 2048] = WT_lo[2]
    cbr2b[:, _O_RWOUT:_O_RWOUT + 256] = woutr

    # fp8 constants
    cb8 = np.zeros((128, CB8_COLS), dtype=E4NP)
    for i in range(3):
        wc = np.zeros((128, 4, 2, 512), dtype=E4NP)
        wc[:, :, 0, :] = _e4(WT_lo[i].reshape(128, 4, 512) * CS)
        wc[:, :, 1, :] = _e4(WT_hi[i].reshape(128, 4, 512))
        cb8[:, _O8_WC[i]:_O8_WC[i] + 4096] = wc.reshape(128, 4096)
    cb8[:, _O8_W81:_O8_W81 + 2048] = _e4(WT[1] * S1)
    cb8[:, _O8_W82:_O8_W82 + 2048] = _e4(WT[2] * S2)
    wout8 = W_out.T.reshape(4, 128, D).transpose(1, 0, 2).reshape(128, 256)
    cb8[:, _O8_WOUT:_O8_WOUT + 256] = _e4(wout8 * SO)
    return cb, cbr1a, cbr1b, cbr2a, cbr2b, cb8


_CACHE = {}


def _get_nc():
    if "nc" not in _CACHE:
        _CACHE["nc"] = _build()
    return _CACHE["nc"]


def kernel(x, W_in, b_in, W_h, b_h, W_out, b_out, _trace=False):
    x = np.asarray(x, dtype=np.float32)
    cb, cbr1a, cbr1b, cbr2a, cbr2b, cb8 = _pack_consts(
        np.asarray(W_in, np.float32), np.asarray(b_in, np.float32),
        np.asarray(W_h, np.float32), np.asarray(b_h, np.float32),
        np.asarray(W_out, np.float32), np.asarray(b_out, np.float32))
    in_maps = []
    for c in range(N_CORES):
        sh = x[c * BC:(c + 1) * BC]          # [512, 192]
        xb = np.ascontiguousarray(
            np.concatenate([sh[:, 0:D].T, sh[:, D:2 * D].T, sh[:, 2 * D:].T],
                           axis=1))           # [64, 1536]
        xbh = _round_fp32r(xb)
        xbl = _round_fp32r(xb - xbh)
        in_maps.append({"cb": cb, "cbr1a": cbr1a, "cbr1b": cbr1b,
                        "cbr2a": cbr2a, "cbr2b": cbr2b, "cb8": cb8,
                        "xbh": xbh, "xbl": xbl})

    nc = _get_nc()
    res = run_bass_kernel_spmd(nc, in_maps, list(range(N_CORES)), trace=_trace)
    out = np.concatenate([res.results[c]["out"] for c in range(N_CORES)], axis=0)
    if _trace:
        kernel._last_results = res
    return out


# revision 15
# speedup vs baseline: 1.3220x; 1.3220x over previous
"""DiffeomorphismNet fused kernel for 8x TRN2 NeuronCores (data parallel).

Math (per sample row x = [xt | xtdot | xz], each 64 wide):
  branch(v):  h0 = W_in v + b_in;  h_{i+1} = relu(W_h[i] h_i + b_h[i]), i=0..2
              D_{i+1} = (W_h[i] h_{i+1} + b_h[i] > 0)        # the module's quirk
  out cols  0:64   h_out  = W_out h3_t + b_out                       (t branch)
  out cols 64:128  h_dot  = W_out D3t W2 D2t W1 D1t (W_h0 W_in) xtdot
  out cols 128:192 zng    = row_norms(W_out D3z W2 D2z W1 D1z G0),  G0 = W_h0 W_in

Precision strategy (validated in a numpy bit-level sim, rel_l2 ~ 7e-3 vs fp64):
  - Forward h-value matmuls: fp32r hi pass + ONE fp8e4 DoubleRow cross pass.
    The cross PSUM holds 2^12 * (Wlo@hhi + Whi@hlo): stationary pairs
    (e4(Wlo*2^12), e4(Whi)) ride DoubleRow against moving pairs (hhi8, hlo8*2^12);
    combine as z = z_hi + 2^-12 * z_cross.  z error ~1e-5 relative.
  - Forward MASK matmuls (sign-critical): 3-pass fp32r hi/lo (err ~2e-7).
  - h0: 3-pass fp32r.  h_out: exact fp32.  h_dot chain: single-pass fp32r.
  - zng Jacobian chain: fp8e4 DoubleRow (K=256/instr = 2x fp32r MAC rate).
    Power-of-2 scales keep fp8 operands in e4m3 normal range:
    G0*8 -> J0; W1*16 -> J1 (sigma~11); W2*2 -> J2 (sigma~16); Wout*16 -> A.
    zng = sqrt(sum A^2 * 2^-24).

Engine balance in the zng phase (PE ~3.8us per group of 8 samples):
  gpsimd: J0 = e4(G0 x D1) build (scalar_tensor_tensor, SBUF-only) + d-reduce;
  DVE: 3 of 4 masked PSUM->fp8 casts per layer;
  ACT: remaining cast as 8 per-sample Copy(scale=mask[p,1]) ops + square + sqrt.

Sharding: batch 4096 -> 8 cores x 512. Weights replicated.
"""

import os
import sys

sys.path.insert(0, "/opt/trn_rl_repo")

import numpy as np
import ml_dtypes
import concourse.bass as bass
import concourse.tile as tile
from concourse import bacc
from concourse import mybir
from concourse.bass_utils import run_bass_kernel_spmd

N_CORES = 8
B = 4096
BC = B // N_CORES          # 512 samples per core
D = 64                     # n
H = 512                    # hidden
NL = 3                     # hidden layers
NMC = H // 128             # partition chunks of the hidden dim
NG = BC // 8               # jacobian groups of 8 samples
JN = 8 * D                 # 512 columns per jacobian group

F32 = mybir.dt.float32
F32R = mybir.dt.float32r
F8 = mybir.dt.float8e4
E4NP = ml_dtypes.float8_e4m3
DR = mybir.MatmulPerfMode.DoubleRow

ADD = mybir.AluOpType.add
MAX = mybir.AluOpType.max
MULT = mybir.AluOpType.mult
SUB = mybir.AluOpType.subtract
ISGT = mybir.AluOpType.is_gt
AF = mybir.ActivationFunctionType

CS = float(2.0 ** 12)      # cross-pass pre-scale
ICS = float(2.0 ** -12)
S0, S1, S2, SO = 8.0, 16.0, 2.0, 16.0     # zng chain scales (product 2^12)
ZS = float(2.0 ** -24)     # sqrt scale = 1/(S0*S1*S2*SO)^2

# cb (fp32) column offsets
_O_WOUTR = 0                 # W_out^T as [128, 4, 64]
_O_BIN = 256                 # [128, 4]
_O_BH = 260                  # [128, 12]
_O_BOUTR = 272               # [128, 64]
_O_ID = 336                  # [128, 128] identity
CB_COLS = 464
# cbr1a (fp32r): layer-0 hi weights + W_in (for G0) + W_in^T hi/lo
_O_RWT0 = 0                  # hi(W_h[0]^T)  [128, 4, 512]
_O_WINR = 2048               # hi(W_in)*S0 as [128, 4, 64]
_O_WINTH = 2304              # hi(W_in^T) rows 0:64, [64, 512]
_O_WINTL = 2816              # lo(W_in^T) rows 0:64, [64, 512]
CBR1A_COLS = 3328
# cbr1b (fp32r): layer-0 lo weights
_O_WLO0 = 0
CBR1B_COLS = 2048
# cbr2a (fp32r): layer 1
_O_RWT1 = 0
_O_WLO1 = 2048
CBR2A_COLS = 4096
# cbr2b (fp32r): layer 2 + W_out^T
_O_RWT2 = 0
_O_WLO2 = 2048
_O_RWOUT = 4096              # hi(W_out^T) as [128, 4, 64]
CBR2B_COLS = 4352
# cb8 (fp8e4): cross-pass stationary pairs + zng chain weights
_O8_WC = [0, 4096, 8192]     # per layer [128, 4kc, 2, 512]: (lo*2^12, hi)
_O8_W81 = 12288              # e4(W_h1^T * S1) [128, 4, 512]
_O8_W82 = 14336              # e4(W_h2^T * S2)
_O8_WOUT = 16384             # e4(W_out^T * SO) [128, 4, 64]
CB8_COLS = 16640


def _round_fp32r(x: np.ndarray) -> np.ndarray:
    """Round-to-nearest-even to 11 explicit mantissa bits (fp32r grid)."""
    u = x.astype(np.float32).view(np.uint32).astype(np.uint64)
    keep = np.uint64(0xFFFFF000)
    half = np.uint64(0x800)
    lsb = (u >> np.uint64(12)) & np.uint64(1)
    r = (u + half - np.uint64(1) + lsb) & keep
    return r.astype(np.uint32).view(np.float32)


def _e4(x: np.ndarray) -> np.ndarray:
    return np.ascontiguousarray(np.asarray(x, np.float32)).astype(E4NP)


def _build():
    nc = bacc.Bacc("TRN2", target_bir_lowering=False, debug=False,
                   num_devices=N_CORES)

    cb_d = nc.dram_tensor("cb", [128, CB_COLS], F32, kind="ExternalInput")
    cbr1a_d = nc.dram_tensor("cbr1a", [128, CBR1A_COLS], F32R, kind="ExternalInput")
    cbr1b_d = nc.dram_tensor("cbr1b", [128, CBR1B_COLS], F32R, kind="ExternalInput")
    cbr2a_d = nc.dram_tensor("cbr2a", [128, CBR2A_COLS], F32R, kind="ExternalInput")
    cbr2b_d = nc.dram_tensor("cbr2b", [128, CBR2B_COLS], F32R, kind="ExternalInput")
    cb8_d = nc.dram_tensor("cb8", [128, CB8_COLS], F8, kind="ExternalInput")
    xbh_d = nc.dram_tensor("xbh", [64, 3 * BC], F32R, kind="ExternalInput")
    xbl_d = nc.dram_tensor("xbl", [64, 3 * BC], F32R, kind="ExternalInput")
    out_d = nc.dram_tensor("out", [BC, 3 * D], F32, kind="ExternalOutput")

    with tile.TileContext(nc) as tc:
        with (
            tc.tile_pool(name="const", bufs=1) as pc,
            tc.tile_pool(name="act", bufs=2) as pa,
            tc.tile_pool(name="mask", bufs=1) as pm,
            tc.tile_pool(name="jac", bufs=3) as pj,
            tc.tile_pool(name="ps", bufs=5, space="PSUM") as ps,
            tc.tile_pool(name="psc", bufs=1, space="PSUM") as psc,
            tc.tile_pool(name="psa", bufs=1, space="PSUM") as psa,
            tc.tile_pool(name="pso", bufs=1, space="PSUM") as pso,
        ):
            cb = pc.tile([128, CB_COLS], F32)
            cbr1a = pc.tile([128, CBR1A_COLS], F32R)
            cbr1b = pc.tile([128, CBR1B_COLS], F32R)
            cbr2a = pc.tile([128, CBR2A_COLS], F32R)
            cbr2b = pc.tile([128, CBR2B_COLS], F32R)
            cb8 = pc.tile([128, CB8_COLS], F8)
            xbh = pc.tile([64, 3 * BC], F32R)
            xbl = pc.tile([64, 3 * BC], F32R)
            nc.sync.dma_start(cbr1a[:], cbr1a_d.ap())
            nc.sync.dma_start(cb[:], cb_d.ap())
            nc.sync.dma_start(xbh[:], xbh_d.ap())
            nc.sync.dma_start(xbl[:], xbl_d.ap())
            nc.sync.dma_start(cbr1b[:], cbr1b_d.ap())
            nc.sync.dma_start(cbr2a[:], cbr2a_d.ap())
            nc.sync.dma_start(cbr2b[:], cbr2b_d.ap())
            nc.sync.dma_start(cb8[:], cb8_d.ap())

            # DVE warm-up: observe each input DMA semaphore once.
            warm = pc.tile([128, 4], F32)
            warm8 = pc.tile([128, 4], F8)
            nc.vector.tensor_copy(warm[0:1, 0:1], cb[0:1, 0:1])
            nc.vector.tensor_copy(warm[0:1, 1:2].bitcast(F32R), cbr1a[0:1, 0:1])
            nc.vector.tensor_copy(warm[0:1, 1:2].bitcast(F32R), cbr1b[0:1, 0:1])
            nc.vector.tensor_copy(warm[0:1, 2:3].bitcast(F32R), cbr2a[0:1, 0:1])
            nc.vector.tensor_copy(warm[0:1, 3:4].bitcast(F32R), cbr2b[0:1, 0:1])
            nc.vector.tensor_copy(warm[0:1, 0:1].bitcast(F32R), xbh[0:1, 0:1])
            nc.vector.tensor_copy(warm[0:1, 1:2].bitcast(F32R), xbl[0:1, 0:1])
            nc.vector.tensor_copy(warm8[0:1, 0:1], cb8[0:1, 0:1])

            WoutR = cb[:, _O_WOUTR:_O_WOUTR + 256].rearrange(
                "p (kc n) -> p kc n", kc=4)
            bin_ = cb[:, _O_BIN:_O_BIN + 4]
            bh = cb[:, _O_BH:_O_BH + 12]
            boutR = cb[:, _O_BOUTR:_O_BOUTR + 64]
            ident = cb[:, _O_ID:_O_ID + 128]

            def r3(ap, off):
                return ap[:, off:off + 2048].rearrange("p (kc m) -> p kc m", kc=4)

            WHI = [r3(cbr1a, _O_RWT0), r3(cbr2a, _O_RWT1), r3(cbr2b, _O_RWT2)]
            WLO = [r3(cbr1b, _O_WLO0), r3(cbr2a, _O_WLO1), r3(cbr2b, _O_WLO2)]
            WinR = cbr1a[:, _O_WINR:_O_WINR + 256].rearrange(
                "p (kc n) -> p kc n", kc=4)
            RWout = cbr2b[:, _O_RWOUT:_O_RWOUT + 256].rearrange(
                "p (kc n) -> p kc n", kc=4)
            WinTH = cbr1a[0:64, _O_WINTH:_O_WINTH + 512]
            WinTL = cbr1a[0:64, _O_WINTL:_O_WINTL + 512]
            WC = [cb8[:, o:o + 4096].rearrange("p (kc t m) -> p kc t m",
                                               kc=4, t=2) for o in _O8_WC]
            W81 = cb8[:, _O8_W81:_O8_W81 + 2048].rearrange(
                "p (kc m) -> p kc m", kc=4)
            W82 = cb8[:, _O8_W82:_O8_W82 + 2048].rearrange(
                "p (kc m) -> p kc m", kc=4)
            Wout8 = cb8[:, _O8_WOUT:_O8_WOUT + 256].rearrange(
                "p (kc n) -> p kc n", kc=4)

            xtTh, xtTl = xbh[:, 0:BC], xbl[:, 0:BC]
            xdTh = xbh[:, BC:2 * BC]
            xzTh, xzTl = xbh[:, 2 * BC:3 * BC], xbl[:, 2 * BC:3 * BC]

            # PE warm-up: observe the cbr1a DMA semaphore from a fresh PSUM slot
            pwarm = ps.tile([128, D], F32, tag="psj1")
            nc.tensor.matmul(pwarm[:], WHI[0][:, 0, 0:128], WHI[0][:, 0, 0:D],
                             start=True, stop=True)

            # ---- G0 = (W_h[0] @ W_in) * S0  [512, 64] as [128, 4(mc), 64] ----
            G0 = pc.tile([128, 4, D], F32)
            for mc in range(NMC):
                pg = pso.tile([128, D], F32, tag="po")
                for kc in range(NMC):
                    nc.tensor.matmul(pg[:], WHI[0][:, kc, mc * 128:(mc + 1) * 128],
                                     WinR[:, kc, :], start=(kc == 0),
                                     stop=(kc == NMC - 1))
                nc.vector.tensor_copy(G0[:, mc, :], pg[:])

            def mm3(psum, i, mc, hhi, hlo):
                """z[mc] += W_h[i] @ h via 3 fp32r passes (12 matmuls)."""
                sl = slice(mc * 128, (mc + 1) * 128)
                n = 0
                for wop, rop in ((WHI[i], hhi), (WLO[i], hhi), (WHI[i], hlo)):
                    for kc in range(NMC):
                        nc.tensor.matmul(psum[:], wop[:, kc, sl], rop[:, kc, :],
                                         start=(n == 0), stop=(n == 11))
                        n += 1

            def forward(xTh, xTl, tagpfx, keep_h3=False):
                """Forward pass; returns (h3_f32|None, masks [D1,D2,D3])."""
                hhi = pa.tile([128, 4, BC], F32R, tag="hhi", name=f"h0h{tagpfx}")
                hlo = pa.tile([128, 4, BC], F32R, tag="hlo", name=f"h0l{tagpfx}")
                h8 = pa.tile([128, 4, 2, BC], F8, tag="h8", name=f"h08{tagpfx}")
                # ---- h0 (3-pass fp32r, no relu) ----
                for mc in range(NMC):
                    p0 = ps.tile([128, BC], F32, tag="psj1", name=f"p0{tagpfx}")
                    sl = slice(mc * 128, (mc + 1) * 128)
                    for n, (w, r) in enumerate(((WinTH, xTh), (WinTL, xTh),
                                                (WinTH, xTl))):
                        nc.tensor.matmul(p0[:], w[:, sl], r,
                                         start=(n == 0), stop=(n == 2))
                    hf = pa.tile([128, BC], F32, tag="hf", name=f"hf0{tagpfx}")
                    nc.vector.tensor_scalar_add(hf[:], p0[:], bin_[:, mc:mc + 1])
                    nc.vector.tensor_copy(hhi[:, mc, :], hf[:])
                    nc.vector.tensor_tensor(hlo[:, mc, :], hf[:],
                                            hhi[:, mc, :].bitcast(F32), SUB)
                    nc.scalar.copy(h8[:, mc, 0, :], hf[:])
                    nc.scalar.activation(h8[:, mc, 1, :],
                                         hlo[:, mc, :].bitcast(F32),
                                         AF.Copy, scale=CS)
                h3f = None
                masks = []
                for i in range(NL):
                    last = (i == NL - 1)
                    hhin = pa.tile([128, 4, BC], F32R, tag="hhi",
                                   name=f"h{i+1}h{tagpfx}")
                    hlon = pa.tile([128, 4, BC], F32R, tag="hlo",
                                   name=f"h{i+1}l{tagpfx}")
                    h8n = pa.tile([128, 4, 2, BC], F8, tag="h8",
                                  name=f"h{i+1}8{tagpfx}")
                    if last and keep_h3:
                        h3f = pa.tile([128, 4, BC], F32, tag="h3f", name="h3f")
                    for mc in range(NMC):
                        sl = slice(mc * 128, (mc + 1) * 128)
                        # fp32r hi pass
                        pz = ps.tile([128, BC], F32, tag="psj1", name=f"pz{tagpfx}")
                        for kc in range(NMC):
                            nc.tensor.matmul(pz[:], WHI[i][:, kc, sl],
                                             hhi[:, kc, :], start=(kc == 0),
                                             stop=(kc == NMC - 1))
                        # fp8 DoubleRow cross pass (2^12-scaled)
                        pcx = psc.tile([128, BC], F32, tag="pc",
                                       name=f"pc{tagpfx}")
                        for kc in range(NMC):
                            nc.tensor.matmul(pcx[:], WC[i][:, kc, :, sl],
                                             h8[:, kc, :, :], start=(kc == 0),
                                             stop=(kc == NMC - 1), perf_mode=DR)
                        c1 = pa.tile([128, BC], F32, tag="c1", name=f"c1{tagpfx}")
                        nc.scalar.activation(c1[:], pcx[:], AF.Copy, scale=ICS)
                        # t = (z_hi + b) + 2^-12 z_cross   (pre-relu)
                        hf = pa.tile([128, BC], F32, tag="hf", name=f"hf{tagpfx}")
                        nc.vector.scalar_tensor_tensor(
                            hf[:], pz[:], bh[:, 4 * i + mc:4 * i + mc + 1],
                            c1[:], ADD, ADD)
                        # splits of h_{i+1} = relu(t)
                        nc.vector.tensor_scalar_max(hhin[:, mc, :], hf[:], 0.0)
                        nc.vector.scalar_tensor_tensor(
                            hlon[:, mc, :], hf[:], 0.0,
                            hhin[:, mc, :].bitcast(F32), MAX, SUB)
                        nc.scalar.activation(h8n[:, mc, 0, :], hf[:], AF.Relu)
                        nc.scalar.activation(h8n[:, mc, 1, :],
                                             hlon[:, mc, :].bitcast(F32),
                                             AF.Copy, scale=CS)
                        if h3f is not None:
                            nc.scalar.activation(h3f[:, mc, :], hf[:], AF.Relu)
                    hhi, hlo, h8 = hhin, hlon, h8n
                    # mask: fp32r hi pass + fp8 DoubleRow cross pass
                    Dm = pm.tile([128, 4, BC], F32, tag=f"D{i}",
                                 name=f"D{i}{tagpfx}")
                    for mc in range(NMC):
                        sl = slice(mc * 128, (mc + 1) * 128)
                        pd = ps.tile([128, BC], F32, tag="psj1", name=f"pd{tagpfx}")
                        for kc in range(NMC):
                            nc.tensor.matmul(pd[:], WHI[i][:, kc, sl],
                                             hhi[:, kc, :], start=(kc == 0),
                                             stop=(kc == NMC - 1))
                        pcm = psc.tile([128, BC], F32, tag="pc",
                                       name=f"pcm{tagpfx}")
                        for kc in range(NMC):
                            nc.tensor.matmul(pcm[:], WC[i][:, kc, :, sl],
                                             h8[:, kc, :, :], start=(kc == 0),
                                             stop=(kc == NMC - 1), perf_mode=DR)
                        cm = pa.tile([128, BC], F32, tag="c1", name=f"cm{tagpfx}")
                        nc.scalar.activation(cm[:], pcm[:], AF.Copy, scale=ICS)
                        tmsk = pa.tile([128, BC], F32, tag="hf",
                                       name=f"tm{tagpfx}")
                        nc.vector.scalar_tensor_tensor(
                            tmsk[:], pd[:], bh[:, 4 * i + mc:4 * i + mc + 1],
                            cm[:], ADD, ADD)
                        nc.vector.tensor_scalar(Dm[:, mc, :], tmsk[:], 0.0,
                                                None, ISGT)
                    masks.append(Dm)
                return h3f, masks

            # ---- output staging tiles, one per 128-sample block ----
            O = [pc.tile([128, 3 * D], F32, tag=f"O{g}", name=f"O{g}")
                 for g in range(4)]

            # ================= t branch =================
            h3t, Dt = forward(xtTh, xtTl, "t", keep_h3=True)

            # h_out (sample-major, exact fp32): out[s,n] = h3t^T W_out^T + b_out
            for mg in range(4):
                po = pso.tile([128, D], F32, tag="po")
                for kc in range(NMC):
                    nc.tensor.matmul(po[:],
                                     h3t[:, kc, mg * 128:(mg + 1) * 128],
                                     WoutR[:, kc, :], start=(kc == 0),
                                     stop=(kc == NMC - 1))
                nc.vector.tensor_add(O[mg][:, 0:D], po[:], boutR)

            # h_dot chain, single-pass fp32r: v = W_h0 (W_in xdT); v = Di*(W v)
            w0r = pa.tile([128, 4, BC], F32R, tag="hhi", name="w0r")
            for mc in range(NMC):
                pw = ps.tile([128, BC], F32, tag="psj1", name="pw")
                nc.tensor.matmul(pw[:], WinTH[:, mc * 128:(mc + 1) * 128], xdTh,
                                 start=True, stop=True)
                nc.vector.tensor_copy(w0r[:, mc, :], pw[:])
            v = w0r
            for i in range(NL):
                vn = pa.tile([128, 4, BC], F32R, tag="hlo", name=f"v{i+1}")
                for mc in range(NMC):
                    pv = ps.tile([128, BC], F32, tag="psj1", name="pv")
                    for kc in range(NMC):
                        nc.tensor.matmul(pv[:],
                                         WHI[i][:, kc, mc * 128:(mc + 1) * 128],
                                         v[:, kc, :], start=(kc == 0),
                                         stop=(kc == NMC - 1))
                    nc.vector.tensor_mul(vn[:, mc, :], pv[:], Dt[i][:, mc, :])
                v = vn
            for mg in range(4):
                po = pso.tile([128, D], F32, tag="po")
                for kc in range(NMC):
                    nc.tensor.matmul(po[:], v[:, kc, mg * 128:(mg + 1) * 128],
                                     RWout[:, kc, :], start=(kc == 0),
                                     stop=(kc == NMC - 1))
                nc.vector.tensor_copy(O[mg][:, D:2 * D], po[:])

            # ================= z branch =================
            _, Dz = forward(xzTh, xzTl, "z")

            zng2 = pc.tile([64, BC], F32)
            zngT = pc.tile([64, BC], F32)

            def zng_flush(mg):
                c0 = mg * 128
                nc.scalar.activation(zngT[:, c0:c0 + 128], zng2[:, c0:c0 + 128],
                                     AF.Sqrt, scale=ZS)
                pt = pso.tile([128, 64], F32, tag="po")
                nc.tensor.transpose(pt[:], zngT[:, c0:c0 + 128],
                                    ident[0:64, 0:64])
                nc.vector.tensor_copy(O[mg][:, 2 * D:3 * D], pt[:])

            # ============ zng: fp8 DoubleRow Jacobian chain ============
            # Software-pipelined across groups: stage s processes group g-s,
            # giving each cross-engine dependency a full iteration of slack
            # (the in-order PE queue otherwise stalls on trailing casts).
            J0r, J1r, J2r, pAr, sqr = {}, {}, {}, {}, {}

            def st_j0(g):
                s0 = g * 8
                J0 = pj.tile([128, 4, 8, D], F8, tag="J0", name="J0")
                nc.gpsimd.tensor_tensor(
                    J0[:],
                    G0[:, :, None, :].broadcast_to([128, 4, 8, D]),
                    Dz[0][:, :, s0:s0 + 8][:, :, :, None]
                    .broadcast_to([128, 4, 8, D]), MULT)
                J0r[g] = J0

            def st_layer(g, i, W8, Jsrc, Jdst):
                s0 = g * 8
                J = Jsrc.pop(g)
                Jn = pj.tile([128, 4, 8, D], F8, tag=f"J{i}", name=f"J{i}")
                for mc in range(NMC):
                    sl = slice(mc * 128, (mc + 1) * 128)
                    pjm = ps.tile([128, JN], F32, tag="psj1", name="pjm")
                    for kp in range(2):
                        nc.tensor.matmul(
                            pjm[:], W8[:, 2 * kp:2 * kp + 2, sl],
                            J[:, 2 * kp:2 * kp + 2, :, :],
                            start=(kp == 0), stop=(kp == 1), perf_mode=DR)
                    if mc == 2 and i == 2:
                        stg = pa.tile([128, JN], F32, tag="stg", name="stg")
                        nc.scalar.copy(stg[:], pjm[:])
                        nc.gpsimd.tensor_tensor(
                            Jn[:, mc, :, :],
                            stg[:].rearrange("p (b d) -> p b d", b=8),
                            Dz[i][:, mc, s0:s0 + 8][:, :, None]
                            .broadcast_to([128, 8, D]), MULT)
                    else:
                        nc.vector.tensor_tensor(
                            Jn[:, mc, :, :],
                            pjm[:].rearrange("p (b d) -> p b d", b=8),
                            Dz[i][:, mc, s0:s0 + 8][:, :, None]
                            .broadcast_to([128, 8, D]), MULT)
                Jdst[g] = Jn

            def st_tail(g):
                s0 = g * 8
                J = J2r.pop(g)
                pA = psa.tile([64, JN], F32, tag="pA")
                for kp in range(2):
                    nc.tensor.matmul(pA[:], Wout8[:, 2 * kp:2 * kp + 2, :],
                                     J[:, 2 * kp:2 * kp + 2, :, :],
                                     start=(kp == 0), stop=(kp == 1),
                                     perf_mode=DR)
                sq = pa.tile([64, JN], F32, tag="sq", name="sq")
                nc.scalar.square(sq[:], pA[:])
                nc.vector.tensor_reduce(
                    zng2[:, s0:s0 + 8],
                    sq[:].rearrange("p (b d) -> p b d", b=8),
                    mybir.AxisListType.X, mybir.AluOpType.add)
                if g % 16 == 15:
                    zng_flush(g // 16)

            for g in range(NG + 3):
                if g < NG:
                    st_j0(g)
                if 1 <= g < NG + 1:
                    st_layer(g - 1, 1, W81, J0r, J1r)
                if 2 <= g < NG + 2:
                    st_layer(g - 2, 2, W82, J1r, J2r)
                if 3 <= g:
                    st_tail(g - 3)

            for mg in range(4):
                nc.sync.dma_start(out_d.ap()[mg * 128:(mg + 1) * 128, :], O[mg][:])

    nc.compile()
    return nc


def _pack_consts(W_in, b_in, W_h, b_h, W_out, b_out):
    cb = np.zeros((128, CB_COLS), dtype=np.float32)
    cb[:, _O_WOUTR:_O_WOUTR + 256] = (
        W_out.T.reshape(4, 128, D).transpose(1, 0, 2).reshape(128, 256))
    cb[:, _O_BIN:_O_BIN + 4] = b_in.reshape(4, 128).T
    cb[:, _O_BH:_O_BH + 12] = b_h.reshape(3, 4, 128).transpose(2, 0, 1).reshape(128, 12)
    cb[:, _O_BOUTR:_O_BOUTR + 64] = np.tile(b_out, (128, 1))
    cb[:, _O_ID:_O_ID + 128] = np.eye(128, dtype=np.float32)

    # W_h[i]^T rearranged to [128, 4, 512] then hi/lo split on the fp32r grid
    WT = np.transpose(W_h, (0, 2, 1)).reshape(3, 4, 128, H).transpose(0, 2, 1, 3)
    WT = WT.reshape(3, 128, 2048)
    WT_hi = _round_fp32r(WT)
    WT_lo = _round_fp32r(WT - WT_hi)

    winr = W_in.reshape(4, 128, D).transpose(1, 0, 2).reshape(128, 256)
    winr = _round_fp32r(winr * S0)
    woutr = W_out.T.reshape(4, 128, D).transpose(1, 0, 2).reshape(128, 256)
    woutr = _round_fp32r(woutr)

    wint = W_in.T
    wint_hi = _round_fp32r(wint)
    wint_lo = _round_fp32r(wint - wint_hi)

    cbr1a = np.zeros((128, CBR1A_COLS), dtype=np.float32)
    cbr1a[:, _O_RWT0:_O_RWT0 + 2048] = WT_hi[0]
    cbr1a[:, _O_WINR:_O_WINR + 256] = winr
    cbr1a[0:64, _O_WINTH:_O_WINTH + 512] = wint_hi
    cbr1a[0:64, _O_WINTL:_O_WINTL + 512] = wint_lo
    cbr1b = np.ascontiguousarray(WT_lo[0])

    cbr2a = np.zeros((128, CBR2A_COLS), dtype=np.float32)
    cbr2a[:, _O_RWT1:_O_RWT1 + 2048] = WT_hi[1]
    cbr2a[:, _O_WLO1:_O_WLO1 + 2048] = WT_lo[1]
    cbr2b = np.zeros((128, CBR2B_COLS), dtype=np.float32)
    cbr2b[:, _O_RWT2:_O_RWT2 + 2048] = WT_hi[2]
    cbr2b[:, _O_WLO2:_O_WLO2 + 2048] = WT_lo[2]
    cbr2b[:, _O_RWOUT:_O_RWOUT + 256] = woutr

    # fp8 constants
    cb8 = np.zeros((128, CB8_COLS), dtype=E4NP)
    for i in range(3):
        wc = np.zeros((128, 4, 2, 512), dtype=E4NP)
        wc[:, :, 0, :] = _e4(WT_lo[i].reshape(128, 4, 512) * CS)
        wc[:, :, 1, :] = _e4(WT_hi[i].reshape(128, 4, 512))
        cb8[:, _O8_WC[i]:_O8_WC[i] + 4096] = wc.reshape(128, 4096)
    cb8[:, _O8_W81:_O8_W81 + 2048] = _e4(WT[1] * S1)
    cb8[:, _O8_W82:_O8_W82 + 2048] = _e4(WT[2] * S2)
    wout8 = W_out.T.reshape(4, 128, D).transpose(1, 0, 2).reshape(128, 256)
    cb8[:, _O8_WOUT:_O8_WOUT + 256] = _e4(wout8 * SO)
    return cb, cbr1a, cbr1b, cbr2a, cbr2b, cb8


_CACHE = {}


def _get_nc():
    if "nc" not in _CACHE:
        _CACHE["nc"] = _build()
    return _CACHE["nc"]


def kernel(x, W_in, b_in, W_h, b_h, W_out, b_out, _trace=False):
    x = np.asarray(x, dtype=np.float32)
    cb, cbr1a, cbr1b, cbr2a, cbr2b, cb8 = _pack_consts(
        np.asarray(W_in, np.float32), np.asarray(b_in, np.float32),
        np.asarray(W_h, np.float32), np.asarray(b_h, np.float32),
        np.asarray(W_out, np.float32), np.asarray(b_out, np.float32))
    in_maps = []
    for c in range(N_CORES):
        sh = x[c * BC:(c + 1) * BC]          # [512, 192]
        xb = np.ascontiguousarray(
            np.concatenate([sh[:, 0:D].T, sh[:, D:2 * D].T, sh[:, 2 * D:].T],
                           axis=1))           # [64, 1536]
        xbh = _round_fp32r(xb)
        xbl = _round_fp32r(xb - xbh)
        in_maps.append({"cb": cb, "cbr1a": cbr1a, "cbr1b": cbr1b,
                        "cbr2a": cbr2a, "cbr2b": cbr2b, "cb8": cb8,
                        "xbh": xbh, "xbl": xbl})

    nc = _get_nc()
    res = run_bass_kernel_spmd(nc, in_maps, list(range(N_CORES)), trace=_trace)
    out = np.concatenate([res.results[c]["out"] for c in range(N_CORES)], axis=0)
    if _trace:
        kernel._last_results = res
    return out
